# revision 10
# baseline (speedup 1.0000x reference)
"""Trainium2 Bass kernel for CapsuleFC EM-routing forward pass.

Shapes: x[256,64,128], current_act[256,64], W[64,128,32,128], num_iter=3.
Outputs: ncv[256,32,128], q[256,64,32], route_class_emb[256,64,32,128].

Strategy: data-parallel over batch across 8 cores (32 b per core), W replicated.
Per core, votes = einsum('bna,namd->bnmd') are computed once by streaming W
(bf16) through the PE and kept entirely in SBUF (bf16, 16.75 MB).  The three
EM-routing iterations then run out of SBUF: logits on DVE (mul + segmented
reduce), softmax on ACT (Exp with fused sum), weighted votes via per-m
tensor_scalar, and the sum over n on the PE (matmul against a 0/1 selector).
route_class_emb is the final weighted votes, cast-DMA'd out as fp32.

Partition layout for votes: partition p = (n mod 4)*32 + b, SBUF tile
g = n // 4 (16 tiles), free dim = (m, d) = 4096.
"""

import math
import numpy as np
import ml_dtypes
from contextlib import ExitStack

import concourse.bass as bass
import concourse.bacc as bacc
import concourse.mybir as mybir
from concourse.tile import TileContext
from concourse.bass_utils import run_bass_kernel_spmd

BF16 = ml_dtypes.bfloat16

B, NIN, DIN, M, DOUT = 256, 64, 128, 32, 128
NCORES = 8
BSH = B // NCORES          # 32 batch elements per core
MD = M * DOUT              # 4096
NT = NIN // 4              # 16 votes tiles (4 n's per tile)
SCALE = 1.0 / math.sqrt(DOUT)

F32 = mybir.dt.float32
BF = mybir.dt.bfloat16


def build_nc(num_iter: int = 3) -> bass.Bass:
    nc = bacc.Bacc()

    xT = nc.dram_tensor("xT", [DIN, NIN * BSH], BF, kind="ExternalInput")
    actt = nc.dram_tensor("act_t", [128, NT], F32, kind="ExternalInput")
    Wt = nc.dram_tensor("Wt", [NIN, DIN, MD], BF, kind="ExternalInput")
    sel4b = nc.dram_tensor("sel4b", [128, 32], BF, kind="ExternalInput")

    ncv_o = nc.dram_tensor("ncv_o", [BSH, MD], F32, kind="ExternalOutput")
    q_o = nc.dram_tensor("q_o", [NT, 128, M], F32, kind="ExternalOutput")
    rce_o = nc.dram_tensor("rce_o", [NT, 128, MD], F32, kind="ExternalOutput")

    W_anm = Wt.rearrange("n a k -> a n k")  # [128, 64, 4096] strided DRAM view

    with TileContext(nc) as tc, ExitStack() as ctx:
        pp = ctx.enter_context(tc.tile_pool(name="persist", bufs=1))
        V = pp.tile([128, NT * MD], BF)            # votes, 128 KB/partition
        xsb = pp.tile([128, NIN * BSH], BF)        # x^T, free = (n, b)
        asb = pp.tile([128, NT], F32)              # act per (partition, tile)
        s4b = pp.tile([128, 32], BF)               # selector: s4b[p, b] = (p%32==b)
        lg = pp.tile([128, NT * M], F32)           # logits, free = (g, m)

        nc.sync.dma_start(out=xsb[:, :], in_=xT[:, :])
        nc.sync.dma_start(out=asb[:, :], in_=actt[:, :])
        nc.sync.dma_start(out=s4b[:, :], in_=sel4b[:, :])

        # ---------------- Phase 1: votes = x @ W, streamed over n ----------
        with tc.tile_pool(name="wstream", bufs=3) as wp, \
             tc.tile_pool(name="p1ps", bufs=2, space="PSUM") as psp:
            for g in range(NT):
                wtiles = []
                for half in range(2):
                    n0 = 4 * g + 2 * half
                    wtile = wp.tile([128, 2 * MD], BF, tag="w", name=f"w_{g}_{half}")
                    for nl in range(2):
                        nc.sync.dma_start(
                            out=wtile[:, nl * MD:(nl + 1) * MD],
                            in_=W_anm[:, n0 + nl, :])
                    wtiles.append(wtile)
                # psA covers (m,d) chunks 0-3, psB chunks 4-7; each is written
                # by all four n's (col groups) of this quad.
                psA = psp.tile([128, 4 * 512], F32, tag="ps", name=f"psA_{g}")
                psB = psp.tile([128, 4 * 512], F32, tag="ps", name=f"psB_{g}")
                for half in range(2):
                    wtile = wtiles[half]
                    for nl in range(2):
                        n = 4 * g + 2 * half + nl
                        j = 2 * half + nl  # col group -> partitions 32j..32j+31
                        lhsT = xsb[:, n * BSH:(n + 1) * BSH]
                        for c in range(8):
                            ps = psA if c < 4 else psB
                            cl = c % 4
                            nc.tensor.matmul(
                                ps[32 * j:32 * j + 32, cl * 512:(cl + 1) * 512],
                                lhsT=lhsT,
                                rhs=wtile[:, nl * MD + c * 512: nl * MD + (c + 1) * 512],
                                start=True, stop=True,
                                tile_position=(0, 32 * j),
                            )
                nc.vector.tensor_copy(
                    V[:, g * MD: g * MD + 2048], psA[:, :])
                nc.scalar.copy(
                    V[:, g * MD + 2048: g * MD + 4096], psB[:, :])

        with tc.tile_pool(name="it", bufs=1) as ip, \
             tc.tile_pool(name="scr", bufs=3) as sp, \
             tc.tile_pool(name="sm", bufs=4) as smp, \
             tc.tile_pool(name="itps", bufs=1, space="PSUM") as ipp:

            # ---------------- ncv_0 = (sum_n V) / M -------------------------
            ncv_ps = ipp.tile([32, MD], F32, tag="ncvps")
            for g in range(NT):
                for c in range(8):
                    nc.tensor.matmul(
                        ncv_ps[:, c * 512:(c + 1) * 512],
                        lhsT=s4b[:, :],
                        rhs=V[:, g * MD + c * 512: g * MD + (c + 1) * 512],
                        start=(g == 0), stop=(g == NT - 1),
                    )

            def broadcast_ncv(ncv_ps, scale):
                # PSUM [32, MD] -> bf16 SBUF [32, MD] -> replicate to 128 parts
                ncv_sb = smp.tile([32, MD], BF, tag="ncvsb", bufs=1)
                nc.scalar.mul(ncv_sb[:, :], ncv_ps[:, :], scale)
                ncvb = ip.tile([128, MD], BF, tag="ncvb", bufs=2)
                for k in range(4):
                    nc.sync.dma_start(out=ncvb[32 * k:32 * (k + 1), :], in_=ncv_sb[:, :])
                return ncvb

            ncvb = broadcast_ncv(ncv_ps, 1.0 / M)

            # ---------------- routing iterations ---------------------------
            for it in range(num_iter):
                last = (it == num_iter - 1)
                ncv_ps = ipp.tile([32, MD], F32, tag="ncvps")
                for g in range(NT):
                    Vg = V[:, g * MD:(g + 1) * MD]
                    lgg = lg[:, g * M:(g + 1) * M]
                    # logits: mul + segmented reduce over d
                    tmp = sp.tile([128, MD], BF, tag="scr")
                    nc.vector.tensor_mul(tmp[:, :], Vg, ncvb[:, :])
                    nc.vector.reduce_sum(
                        lgg, tmp.rearrange("p (m d) -> p m d", d=DOUT),
                        axis=mybir.AxisListType.X)
                    # softmax over m (free dim)
                    mx = smp.tile([128, 1], F32, tag="mx")
                    nc.vector.reduce_max(mx[:, :], lgg, axis=mybir.AxisListType.X)
                    nmx = smp.tile([128, 1], F32, tag="nmx")
                    nc.vector.tensor_scalar_mul(nmx[:, :], mx[:, :], -SCALE)
                    eqf = smp.tile([128, M], F32, tag="eqf")
                    se = smp.tile([128, 1], F32, tag="se")
                    nc.scalar.activation(
                        eqf[:, :], lgg, mybir.ActivationFunctionType.Exp,
                        bias=nmx[:, 0:1], scale=SCALE, accum_out=se[:, 0:1])
                    rc = smp.tile([128, 1], F32, tag="rc")
                    nc.vector.reciprocal(rc[:, :], se[:, :])
                    wg = smp.tile([128, M], F32, tag="wg")
                    nc.vector.tensor_scalar(
                        wg[:, :], eqf[:, :], rc[:, 0:1], asb[:, g:g + 1],
                        op0=mybir.AluOpType.mult, op1=mybir.AluOpType.mult)
                    if last:
                        qg = smp.tile([128, M], F32, tag="qg")
                        nc.vector.tensor_scalar_mul(qg[:, :], eqf[:, :], rc[:, 0:1])
                        nc.sync.dma_start(out=q_o[g], in_=qg[:, :])
                    # weighted votes rw = V * w (broadcast w over d)
                    rw = sp.tile([128, MD], BF, tag="scr")
                    for m in range(M):
                        nc.vector.tensor_scalar_mul(
                            rw[:, m * DOUT:(m + 1) * DOUT],
                            Vg[:, m * DOUT:(m + 1) * DOUT], wg[:, m:m + 1])
                    for c in range(8):
                        nc.tensor.matmul(
                            ncv_ps[:, c * 512:(c + 1) * 512],
                            lhsT=s4b[:, :],
                            rhs=rw[:, c * 512:(c + 1) * 512],
                            start=(g == 0), stop=(g == NT - 1),
                        )
                    if last:
                        # route_class_emb shard: cast bf16 -> f32 during DMA
                        nc.gpsimd.dma_start(out=rce_o[g], in_=rw[:, :])
                if not last:
                    ncvb = broadcast_ncv(ncv_ps, 1.0)
                else:
                    ncv_f = sp.tile([128, MD], BF, tag="scr", name="ncv_f")
                    nc.scalar.copy(ncv_f[0:BSH, :], ncv_ps[:, :])
                    nc.gpsimd.dma_start(out=ncv_o[:, :], in_=ncv_f[0:BSH, :])

    nc.compile()
    return nc


_NC_CACHE: dict = {}


def _get_nc(num_iter: int) -> bass.Bass:
    if num_iter not in _NC_CACHE:
        _NC_CACHE[num_iter] = build_nc(num_iter)
    return _NC_CACHE[num_iter]


def _host_prep(x, current_act, W):
    x = np.asarray(x, dtype=np.float32)
    act = np.asarray(current_act, dtype=np.float32)
    W = np.asarray(W, dtype=np.float32)

    Wt = np.ascontiguousarray(W.reshape(NIN, DIN, MD).astype(BF16))

    sel4b = np.zeros((128, 32), dtype=BF16)
    for p in range(128):
        sel4b[p, p % 32] = 1

    in_maps = []
    for core in range(NCORES):
        xs = x[core * BSH:(core + 1) * BSH]          # [32, 64, 128]
        acts = act[core * BSH:(core + 1) * BSH]      # [32, 64]
        # xT[a, (n, b)] = x[b, n, a]
        xT = np.ascontiguousarray(
            xs.transpose(2, 1, 0).reshape(DIN, NIN * BSH).astype(BF16))
        # act_t[p, g] = act[b, n] with b = p%32, n = 4g + p//32
        p = np.arange(128)
        g = np.arange(NT)
        act_t = np.ascontiguousarray(
            acts[p[:, None] % 32, 4 * g[None, :] + p[:, None] // 32]
            .astype(np.float32))
        in_maps.append({
            "xT": xT, "act_t": act_t, "Wt": Wt, "sel4b": sel4b,
        })
    return in_maps


def _host_post(results):
    ncv = np.empty((B, M, DOUT), dtype=np.float32)
    q = np.empty((B, NIN, M), dtype=np.float32)
    rce = np.empty((B, NIN, M, DOUT), dtype=np.float32)
    for core, res in enumerate(results):
        sl = slice(core * BSH, (core + 1) * BSH)
        ncv[sl] = res["ncv_o"].reshape(BSH, M, DOUT)
        # q_o[g, p, m] -> q[b, 4g + p//32, m], b = p%32
        q_t = res["q_o"].reshape(NT, 4, 32, M)          # [g, pn, b, m]
        q[sl] = q_t.transpose(2, 0, 1, 3).reshape(BSH, NIN, M)
        rce_t = res["rce_o"].reshape(NT, 4, 32, M, DOUT)  # [g, pn, b, m, d]
        rce[sl] = rce_t.transpose(2, 0, 1, 3, 4).reshape(BSH, NIN, M, DOUT)
    return ncv, q, rce


def kernel(x, current_act, W, num_iter=3, _trace=False, _tmpdir=None):
    num_iter = max(1, int(num_iter))
    nc = _get_nc(num_iter)
    in_maps = _host_prep(x, current_act, W)
    res = run_bass_kernel_spmd(
        nc, in_maps, list(range(NCORES)),
        trace=_trace, tmpdir=_tmpdir)
    out = _host_post(res.results)
    if _trace:
        return out, res
    return out
